# revision 19
# baseline (speedup 1.0000x reference)
"""Trainium2 Bass kernel for nn_Camada_33612414059004.

Computes, for x:[B,N,D,S], M:[N,N], w_syn:[N,D,S], b_dend:[N,D],
w_dend:[N,D], b_soma:[N]:

    xm    = einsum('bids,oi->bods', x, M)
    dend  = tanh(einsum('bnds,nds->bnd', xm, w_syn) + b_dend)
    soma  = einsum('bnd,nd->bn', dend, w_dend) + b_soma
    out   = sigmoid(soma)                                  # [B, N]

Sharding: data-parallel over batch across 8 NeuronCores (B=64 -> 8/core),
zero cross-core communication.  Per core the dominant work is the
connectivity matmul  M[o,i] @ x[i, (b,d,s)]  ([1024x1024]x[1024x1024]).

Measured-rate design (all timings from neuron-profile traces):
 - fp8(e4m3) matmul operands with perf_mode=DoubleRow: M is an exact 0/1
   matrix (fp8-lossless) and x quantization costs ~0.5% final rel-err.
   DoubleRow packs 2 fp8 weights per PE cell -> 256-deep contraction per
   matmul, measured 216ns per 512-wide matmul (~2x bf16 FLOP rate), and
   the input DMA bytes halve.
 - Postprocess all-fp32 (Q7/GpSimd runs bf16 ~3x slower; DVE fast modes
   never engage, so fp32 at 1x is the best case): PSUM drains (the only
   PSUM-capable engines are DVE and ACT) split DVE tensor_mul / ACT Copy
   + GpSimd multiply; s-reduces as DVE tensor_reduce batches or GpSimd
   pairwise trees; bias/soma/tanh/sigmoid batched per tile-group with
   b_soma folded in before the (batched) sigmoid.
 - ACT function table preloaded with dummy activations at kernel start
   (first-use table load measured 1.3us).
 - Params travel on the scalar HWDGE queue interleaved with the mt pairs
   (the gpsimd SWDGE path measured ~5x slower and gated the drains).
 - PE warm-up: staggered dummy matmuls bridge the first-input DMA wait
   so the HAM clock-gate window (3.4us) never sees an idle gap and real
   matmuls run at 2.4GHz from the start.
 - Last o-tile runs half-width matmuls and two independent all-DVE
   latency chains (per batch half), h0 postprocess overlapping h1
   matmuls.  Output DMAs ride the otherwise-idle Sync queue.
"""

import numpy as np
import ml_dtypes
from contextlib import ExitStack

import concourse.bass as bass
import concourse.mybir as mybir
import concourse.tile as tile

B, N, D, S = 64, 1024, 8, 16
NCORES = 8
BC = B // NCORES          # batches per core = 8
DS = D * S                # 128
P = 128                   # SBUF partitions
KP = 4                    # contraction pair-steps (256 input neurons each)
OT = N // P               # 8 output-neuron tiles
FH = 512                  # one fp32 PSUM bank of moving free dim
FW = 2 * FH               # full o-tile moving width (2 banks)
BD = BC * D               # 64
GRP = 4                   # o-tiles in the pair-outer leading group
B0, W1, B1 = 0, OT * D, 2 * OT * D      # smallp cols: b_dend | w_dend | b_soma
SPC = 2 * OT * D + OT                   # 136

F32 = mybir.dt.float32
BF16 = mybir.dt.bfloat16
F8 = mybir.dt.float8e4

_NC_CACHE = {}


def legalize_waits(nc, max_attached=1):
    """Split multi-semaphore waits onto preceding same-engine NOPs.

    The walrus build in this environment accepts at most one sync-wait
    command per instruction (setupSyncWait: "Too many sync wait commands"),
    but Tile attaches one wait per out-of-date engine clock.  An engine is
    in-order, so hoisting the extra waits onto NOPs immediately before the
    instruction is semantics-preserving.
    """
    nid = 0
    for f in nc.m.functions:
        for blk in f.blocks:
            new = []
            changed = False
            for inst in blk.instructions:
                si = inst.sync_info
                if si is not None and si.on_wait and len(si.on_wait) > max_attached:
                    waits = list(si.on_wait)
                    for w in waits[:-max_attached]:
                        nid += 1
                        nop = mybir.InstNoOp(name=f"WSPLIT-{nid}", ins=[], outs=[])
                        nop.engine = inst.engine
                        nop.sync_info = mybir.SyncInfo(on_wait=[w], on_update=[])
                        new.append(nop)
                    inst.sync_info = mybir.SyncInfo(
                        on_wait=waits[-max_attached:], on_update=list(si.on_update)
                    )
                    changed = True
                new.append(inst)
            if changed:
                blk.instructions = new
    return nc


def build_nc(legalize=True):
    """Build the single-core Bass program (SPMD: same program on all cores)."""
    nc = bass.Bass()
    # mt cols: (o-tile t, pair-member j, o-within-tile) so per-o-tile lhsT
    # slices and the tile-0-first DMA split are both contiguous.
    mt = nc.declare_dram_parameter("mt", [KP * P, OT * 2 * P], F8, isOutput=False)
    # xc cols: (pair-member j, (b, d, s)).
    xc = nc.declare_dram_parameter("xc", [KP * P, 2 * FW], F8, isOutput=False)
    wsyn = nc.declare_dram_parameter("wsyn", [P, OT * DS], BF16, isOutput=False)
    smallp = nc.declare_dram_parameter("smallp", [P, SPC], F32, isOutput=False)
    out = nc.declare_dram_parameter("out", [P, OT * BC], F32, isOutput=True)

    AF = mybir.ActivationFunctionType
    AX = mybir.AxisListType
    OP = mybir.AluOpType
    DR = mybir.MatmulPerfMode.DoubleRow

    with tile.TileContext(nc) as tc, ExitStack() as ctx:
        wpool = ctx.enter_context(tc.tile_pool(name="weights", bufs=1))
        xpool = ctx.enter_context(tc.tile_pool(name="xin", bufs=1))
        pspool = ctx.enter_context(tc.tile_pool(name="ps", bufs=4, space="PSUM"))
        smpool = ctx.enter_context(tc.tile_pool(name="smp", bufs=2))

        # --- PE pre-warm + ACT table preload while the first input chunk
        # is in flight.  Staggered dummies (short then long) keep the PE
        # active from ~8us until the first data lands ~12us, so the HAM
        # clock-gate lifts to 2.4GHz and never drops back. ---
        warm_sb = wpool.tile([P, FH], BF16, tag="warm", name="warm_sb")
        nc.gpsimd.memset(warm_sb[:], 0.0)
        preld = wpool.tile([P, 2], F32, tag="preld", name="preld")
        nc.scalar.activation(preld[:, 0:1], warm_sb[:, 0:1], AF.Tanh)
        nc.scalar.activation(preld[:, 1:2], warm_sb[:, 0:1], AF.Sigmoid)
        warm_ps = pspool.tile([P, FW], F32, tag="ps", name="warm_ps")
        for _ in range(8):
            nc.tensor.matmul(
                warm_ps[:, 0:P], lhsT=warm_sb[:, 0:P], rhs=warm_sb[:, 0:P],
                start=True, stop=True,
            )
        for _ in range(5):
            nc.tensor.matmul(
                warm_ps[:, 0:FH], lhsT=warm_sb[:, 0:P], rhs=warm_sb[:],
                start=True, stop=True,
            )

        # --- input DMAs: x on Sync HWDGE, mt+params on Scalar HWDGE
        # (parallel rings).  Pair 0 split so the first matmul starts after
        # ~160KB; params interleaved so they land before the first drain
        # without delaying the pair stream's critical chunks. ---
        x_tiles, mt_tiles = [], []
        for p in range(KP):
            xt = xpool.tile([P, 2 * FW], F8, tag=f"x{p}", name=f"x{p}")
            mtp = xpool.tile([P, OT * 2 * P], F8, tag=f"m{p}", name=f"m{p}")
            x_tiles.append(xt)
            mt_tiles.append(mtp)
        smallp_sb = wpool.tile([P, SPC], F32, tag="smallp", name="smallp_sb")
        wsyn_sb = wpool.tile([P, OT * DS], BF16, tag="wsyn", name="wsyn_sb")

        nc.sync.dma_start(x_tiles[0][:, 0:FW], xc[0:P, 0:FW])
        nc.sync.dma_start(x_tiles[0][:, FW:], xc[0:P, FW:])
        nc.scalar.dma_start(mt_tiles[0][:, 0:2 * P], mt[0:P, 0:2 * P])
        nc.scalar.dma_start(mt_tiles[0][:, 2 * P:], mt[0:P, 2 * P:])
        for p in range(1, KP):
            nc.sync.dma_start(x_tiles[p][:], xc[p * P:(p + 1) * P, :])
            nc.scalar.dma_start(mt_tiles[p][:], mt[p * P:(p + 1) * P, :])
            if p == 1:
                nc.scalar.dma_start(smallp_sb[:], smallp[:, :])
            elif p == 2:
                nc.sync.dma_start(wsyn_sb[:], wsyn[:, :])

        # bf16 prod for the DVE path (halves the SBUF multiply cost via the
        # 2x packed mode); separate fp32 prods for the GpSimd-multiplied
        # tiles (Q7 runs bf16 ~3x slower than fp32).
        prod_all = wpool.tile([P, OT * BC * DS], BF16, tag="prod", name="prod_all")
        prod3 = wpool.tile([P, BC * DS], F32, tag="prod3", name="prod3")
        prod5 = wpool.tile([P, BC * DS], F32, tag="prod5", name="prod5")
        prod6 = wpool.tile([P, BC * DS], F32, tag="prod6", name="prod6")
        dp_all = wpool.tile([P, OT * BD], F32, tag="dp", name="dp_all")
        dend_all = wpool.tile([P, OT * BD], F32, tag="dend", name="dend_all")
        soma_all = wpool.tile([P, OT * BC], F32, tag="soma", name="soma_all")
        out_sb = wpool.tile([P, OT * BC], F32, tag="out", name="out_sb")

        def mm_half(pst, t, p, h):
            nc.tensor.matmul(
                pst[:, h * FH:(h + 1) * FH],
                lhsT=mt_tiles[p][:, t * 2 * P:(t + 1) * 2 * P]
                .rearrange("p (j o) -> p j o", j=2),
                rhs=x_tiles[p][:].rearrange("p (j c) -> p j c", j=2)
                [:, :, h * FH:(h + 1) * FH],
                start=(p == 0), stop=(p == KP - 1), perf_mode=DR,
            )

        def wsyn_bc(t, b):
            return (wsyn_sb[:, t * DS:(t + 1) * DS].unsqueeze(1)
                    .broadcast_to([P, b, DS]))

        def prslice(t, b=BC, h=0):
            c0 = t * BC * DS + h * (BC // 2) * DS
            return prod_all[:, c0:c0 + b * DS]

        def drain_dve(t, pst, b=BC, h=0):
            # prod[o, b, (d,s)] = psum * w_syn (broadcast over b)
            nc.vector.tensor_mul(
                prslice(t, b, h).rearrange("p (b q) -> p b q", b=b),
                pst.rearrange("p (b q) -> p b q", b=b),
                wsyn_bc(t, b),
            )

        def act_copy(t, pst, cpy):
            # ACT copies PSUM->SBUF (it is the only PSUM-capable engine
            # besides DVE, and this frees the banks early for the trailing
            # tiles' matmuls).
            nc.scalar.activation(cpy[:], pst, AF.Copy)

        def mult_dve(t, cpy):
            # bf16 x bf16 -> bf16 tensor_tensor qualifies for the DVE 2x
            # packed mode.
            nc.vector.tensor_mul(
                prslice(t).rearrange("p (b q) -> p b q", b=BC),
                cpy[:].rearrange("p (b q) -> p b q", b=BC),
                wsyn_bc(t, BC),
            )

        def mult_gps(t, cpy, pr):
            nc.gpsimd.tensor_mul(
                pr[:].rearrange("p (b q) -> p b q", b=BC),
                cpy[:].rearrange("p (b q) -> p b q", b=BC),
                wsyn_bc(t, BC),
            )

        def sred_dve(t_lo, nt, src=None):
            if src is None:
                src = (prod_all[:, t_lo * BC * DS:(t_lo + nt) * BC * DS]
                       .rearrange("p (bd s) -> p bd s", s=S))
            nc.vector.tensor_reduce(
                dp_all[:, t_lo * BD:(t_lo + nt) * BD],
                src, axis=AX.X, op=OP.add,
            )

        def sred_gps(tag, t, pr):
            # single-tile fp32 pairwise tree on GpSimd (no free-dim
            # tensor_reduce on Q7; fp32 is its only fast dtype)
            pv = pr[:].rearrange("p (bd s) -> p bd s", s=S)
            g1 = smpool.tile([P, BD * 8], F32, tag=f"{tag}g1", name=f"{tag}g1")
            g2 = smpool.tile([P, BD * 4], F32, tag=f"{tag}g2", name=f"{tag}g2")
            g3 = smpool.tile([P, BD * 2], F32, tag=f"{tag}g3", name=f"{tag}g3")
            g1v = g1[:].rearrange("p (bd s) -> p bd s", s=8)
            g2v = g2[:].rearrange("p (bd s) -> p bd s", s=4)
            g3v = g3[:].rearrange("p (bd s) -> p bd s", s=2)
            nc.gpsimd.tensor_add(g1v, pv[:, :, 0:8], pv[:, :, 8:16])
            nc.gpsimd.tensor_add(g2v, g1v[:, :, 0:4], g1v[:, :, 4:8])
            nc.gpsimd.tensor_add(g3v, g2v[:, :, 0:2], g2v[:, :, 2:4])
            nc.gpsimd.tensor_add(
                dp_all[:, t * BD:(t + 1) * BD].unsqueeze(2),
                g3v[:, :, 0:1], g3v[:, :, 1:2])

        def bias_dve(t_lo, nt):
            dpv = (dp_all[:, t_lo * BD:(t_lo + nt) * BD]
                   .rearrange("p (t b d) -> p t b d", b=BC, d=D))
            nc.vector.tensor_add(
                dpv, dpv,
                smallp_sb[:, B0 + t_lo * D:B0 + (t_lo + nt) * D]
                .rearrange("p (t d) -> p t d", d=D).unsqueeze(2)
                .broadcast_to([P, nt, BC, D]))

        def tanh_batch(t_lo, t_hi):
            nc.scalar.activation(
                dend_all[:, t_lo * BD:t_hi * BD],
                dp_all[:, t_lo * BD:t_hi * BD], AF.Tanh)

        def soma_dve(t_lo, nt):
            # sp = dend * w_dend (broadcast over b); d-reduce; + b_soma
            # (folded so the sigmoids can batch across tiles).
            n = nt * BD
            sp = smpool.tile([P, n], F32, tag="sp", name=f"sp{t_lo}")
            nc.vector.tensor_mul(
                sp[:].rearrange("p (t b d) -> p t b d", b=BC, d=D),
                dend_all[:, t_lo * BD:(t_lo + nt) * BD]
                .rearrange("p (t b d) -> p t b d", b=BC, d=D),
                smallp_sb[:, W1 + t_lo * D:W1 + (t_lo + nt) * D]
                .rearrange("p (t d) -> p t d", d=D).unsqueeze(2)
                .broadcast_to([P, nt, BC, D]))
            ssl = slice(t_lo * BC, (t_lo + nt) * BC)
            nc.vector.tensor_reduce(
                soma_all[:, ssl],
                sp[:].rearrange("p (tb d) -> p tb d", d=D),
                axis=AX.X, op=OP.add)
            nc.vector.tensor_add(
                soma_all[:, ssl].rearrange("p (t b) -> p t b", b=BC),
                soma_all[:, ssl].rearrange("p (t b) -> p t b", b=BC),
                smallp_sb[:, B1 + t_lo:B1 + t_lo + nt].unsqueeze(2)
                .broadcast_to([P, nt, BC]))

        def sigmoid_batch(t_lo, t_hi):
            nc.scalar.activation(
                out_sb[:, t_lo * BC:t_hi * BC],
                soma_all[:, t_lo * BC:t_hi * BC], AF.Sigmoid)

        # ---- matmuls ----
        pst = {}
        for t in range(GRP):
            pst[t] = pspool.tile([P, FW], F32, tag="ps", name=f"ps{t}")
        for p in range(KP):
            for t in range(GRP):
                mm_half(pst[t], t, p, 0)
                mm_half(pst[t], t, p, 1)
        for t in range(GRP, OT):
            pst[t] = pspool.tile([P, FW], F32, tag="ps", name=f"ps{t}")
            if t < OT - 1:
                for p in range(KP):
                    mm_half(pst[t], t, p, 0)
                    mm_half(pst[t], t, p, 1)
            else:
                for h in range(2):
                    for p in range(KP):
                        mm_half(pst[t], t, p, h)

        # ---- postprocess ----
        # (Tile's per-engine scheduler reorders by readiness, so issue
        # order here only fixes relative priority.)
        cp1 = smpool.tile([P, BC * DS], BF16, tag="cpb", name="cp1", bufs=2)
        cp3 = smpool.tile([P, BC * DS], F32, tag="cpf", name="cp3", bufs=2)
        cp5 = smpool.tile([P, BC * DS], F32, tag="cpf", name="cp5", bufs=2)
        cp6 = smpool.tile([P, BC * DS], F32, tag="cpf", name="cp6", bufs=2)
        # Leading tiles 0-3: even tiles DVE-drained; odd tiles ACT-copied
        # (bf16) then DVE-multiplied at the 2x packed rate.  Everything
        # downstream of the copies stays on DVE, so no slow-engine
        # coupling inside the lead group.
        drain_dve(0, pst[0][:])
        act_copy(1, pst[1][:], cp1)
        drain_dve(2, pst[2][:])
        act_copy(3, pst[3][:], cp3)
        mult_dve(1, cp1)
        mult_gps(3, cp3, prod3)
        sred_gps("w", 3, prod3)
        sred_dve(0, 2)
        sred_dve(2, 1)
        bias_dve(0, 4)
        tanh_batch(0, 4)
        soma_dve(0, 4)
        sigmoid_batch(0, 4)
        # Trailing tile 4: ACT-copied, bf16 DVE multiply, DVE reduce.
        cp4 = smpool.tile([P, BC * DS], BF16, tag="cpb", name="cp4", bufs=2)
        act_copy(4, pst[4][:], cp4)
        mult_dve(4, cp4)
        sred_dve(4, 1)
        bias_dve(4, 1)
        tanh_batch(4, 5)
        soma_dve(4, 1)
        sigmoid_batch(4, 5)
        nc.sync.dma_start(out[:, 0:5 * BC], out_sb[:, 0:5 * BC])
        # Tiles 5-6 are ACT-copied now (frees their PSUM banks) but their
        # slow GpSimd pipelines are ISSUED AFTER tile 7's latency chains:
        # the static per-engine schedule follows program order, so nothing
        # that waits on GpSimd may precede the critical final chain.
        act_copy(5, pst[5][:], cp5)
        act_copy(6, pst[6][:], cp6)
        # Last tile: two independent all-DVE latency chains (one per
        # PSUM half = batch half).
        t = OT - 1
        HB = BC // 2          # 4 batches per half
        for h in range(2):
            drain_dve(t, pst[t][:, h * FH:(h + 1) * FH], b=HB, h=h)
            dsl = slice(t * BD + h * HB * D, t * BD + (h + 1) * HB * D)
            nc.vector.tensor_reduce(
                dp_all[:, dsl],
                prslice(t, HB, h).rearrange("p (bd s) -> p bd s", s=S),
                axis=AX.X, op=OP.add)
            nc.vector.tensor_add(
                dp_all[:, dsl].rearrange("p (b d) -> p b d", d=D),
                dp_all[:, dsl].rearrange("p (b d) -> p b d", d=D),
                smallp_sb[:, B0 + t * D:B0 + (t + 1) * D].unsqueeze(1)
                .broadcast_to([P, HB, D]))
            nc.scalar.activation(dend_all[:, dsl], dp_all[:, dsl], AF.Tanh)
            sp7 = smpool.tile([P, HB * D], F32, tag="sp7", name=f"sp7{h}")
            nc.vector.tensor_mul(
                sp7[:].rearrange("p (b d) -> p b d", d=D),
                dend_all[:, dsl].rearrange("p (b d) -> p b d", d=D),
                smallp_sb[:, W1 + t * D:W1 + (t + 1) * D].unsqueeze(1)
                .broadcast_to([P, HB, D]))
            ssl = slice(t * BC + h * HB, t * BC + (h + 1) * HB)
            nc.vector.tensor_reduce(
                soma_all[:, ssl],
                sp7[:].rearrange("p (b d) -> p b d", d=D),
                axis=AX.X, op=OP.add)
            nc.vector.tensor_add(
                soma_all[:, ssl], soma_all[:, ssl],
                smallp_sb[:, B1 + t:B1 + t + 1].broadcast_to([P, HB]))
            nc.scalar.activation(out_sb[:, ssl], soma_all[:, ssl], AF.Sigmoid)

        nc.sync.dma_start(out[:, 7 * BC:], out_sb[:, 7 * BC:])
        # Tiles 5-6 postprocess (GpSimd multiply + tree, fp32 its fast
        # dtype) — off the critical path, only their own out DMA waits.
        mult_gps(5, cp5, prod5)
        sred_gps("u", 5, prod5)
        bias_dve(5, 1)
        tanh_batch(5, 6)
        soma_dve(5, 1)
        mult_gps(6, cp6, prod6)
        sred_gps("v", 6, prod6)
        bias_dve(6, 1)
        tanh_batch(6, 7)
        soma_dve(6, 1)
        sigmoid_batch(5, 7)
        nc.sync.dma_start(out[:, 5 * BC:7 * BC], out_sb[:, 5 * BC:7 * BC])

    if legalize:
        legalize_waits(nc)
    return nc


def get_nc():
    if "nc" not in _NC_CACHE:
        _NC_CACHE["nc"] = build_nc()
    return _NC_CACHE["nc"]


def pack_static(matriz_conexao, w_syn, b_dend, w_dend, b_soma):
    """Pack the batch-independent operands (shared by all cores)."""
    # mt rows (pair p, r), cols (t, j, o): lhsT[r, j, o] = M[t*128+o, (2p+j)*128+r]
    mtT = np.ascontiguousarray(np.asarray(matriz_conexao, np.float32).T)  # [i, o]
    mt_np = (mtT.reshape(KP, 2, P, OT, P)        # [p, j, r, t, o]
             .transpose(0, 2, 3, 1, 4)           # [p, r, t, j, o]
             .reshape(KP * P, OT * 2 * P)
             .astype(ml_dtypes.float8_e4m3))
    ws = (np.asarray(w_syn, np.float32).reshape(OT, P, DS).transpose(1, 0, 2)
          .reshape(P, OT * DS).astype(ml_dtypes.bfloat16))
    bd = np.asarray(b_dend, np.float32).reshape(OT, P, D).transpose(1, 0, 2).reshape(P, OT * D)
    wd = np.asarray(w_dend, np.float32).reshape(OT, P, D).transpose(1, 0, 2).reshape(P, OT * D)
    bs = np.asarray(b_soma, np.float32).reshape(OT, P).T
    smallp_np = np.ascontiguousarray(
        np.concatenate([bd, wd, bs], axis=1).astype(np.float32))
    return mt_np, np.ascontiguousarray(ws), smallp_np


def prepare_in_maps(x, matriz_conexao, w_syn, b_dend, w_dend, b_soma):
    mt_np, ws_np, smallp_np = pack_static(matriz_conexao, w_syn, b_dend, w_dend, b_soma)
    x = np.asarray(x, np.float32)
    xq = x.astype(ml_dtypes.float8_e4m3)
    # xt[i, b, (d,s)] then per core rows (pair p, r), cols (j, b, d, s)
    xt = np.ascontiguousarray(xq.transpose(1, 0, 2, 3).reshape(N, B, DS))
    in_maps = []
    for c in range(NCORES):
        xcor = xt[:, c * BC:(c + 1) * BC, :]          # [N, 8, 128]
        xc_np = np.ascontiguousarray(
            xcor.reshape(KP, 2, P, BC * DS)            # [p, j, r, c]
            .transpose(0, 2, 1, 3)                     # [p, r, j, c]
            .reshape(KP * P, 2 * FW))
        in_maps.append({"mt": mt_np, "xc": xc_np,
                        "wsyn": ws_np, "smallp": smallp_np})
    return in_maps


def assemble_output(results):
    outs = []
    for c in range(NCORES):
        oc = np.asarray(results[c]["out"])          # [P, (t, b)]
        outs.append(oc.reshape(P, OT, BC).transpose(2, 1, 0).reshape(BC, N))
    return np.ascontiguousarray(np.concatenate(outs, axis=0).astype(np.float32))


def kernel(x, matriz_conexao, w_syn, b_dend, w_dend, b_soma):
    from concourse.bass_utils import run_bass_kernel_spmd
    in_maps = prepare_in_maps(x, matriz_conexao, w_syn, b_dend, w_dend, b_soma)
    nc = get_nc()
    res = run_bass_kernel_spmd(nc, in_maps, list(range(NCORES)))
    return assemble_output(res.results)


# revision 21
# speedup vs baseline: 1.0668x; 1.0668x over previous
"""Trainium2 Bass kernel for nn_Camada_33612414059004.

Computes, for x:[B,N,D,S], M:[N,N], w_syn:[N,D,S], b_dend:[N,D],
w_dend:[N,D], b_soma:[N]:

    xm    = einsum('bids,oi->bods', x, M)
    dend  = tanh(einsum('bnds,nds->bnd', xm, w_syn) + b_dend)
    soma  = einsum('bnd,nd->bn', dend, w_dend) + b_soma
    out   = sigmoid(soma)                                  # [B, N]

Sharding: data-parallel over batch across 8 NeuronCores (B=64 -> 8/core),
zero cross-core communication.  Per core the dominant work is the
connectivity matmul  M[o,i] @ x[i, (b,d,s)]  ([1024x1024]x[1024x1024]).

Measured-rate design (all timings from neuron-profile traces):
 - fp8(e4m3) matmul operands with perf_mode=DoubleRow: M is an exact 0/1
   matrix (fp8-lossless) and x quantization costs ~0.5% final rel-err.
   DoubleRow packs 2 fp8 weights per PE cell -> 256-deep contraction per
   matmul, measured 216ns per 512-wide matmul (~2x bf16 FLOP rate), and
   the input DMA bytes halve.
 - Postprocess all-fp32 (Q7/GpSimd runs bf16 ~3x slower; DVE fast modes
   never engage, so fp32 at 1x is the best case): PSUM drains (the only
   PSUM-capable engines are DVE and ACT) split DVE tensor_mul / ACT Copy
   + GpSimd multiply; s-reduces as DVE tensor_reduce batches or GpSimd
   pairwise trees; bias/soma/tanh/sigmoid batched per tile-group with
   b_soma folded in before the (batched) sigmoid.
 - ACT function table preloaded with dummy activations at kernel start
   (first-use table load measured 1.3us).
 - Params travel on the scalar HWDGE queue interleaved with the mt pairs
   (the gpsimd SWDGE path measured ~5x slower and gated the drains).
 - PE warm-up: staggered dummy matmuls bridge the first-input DMA wait
   so the HAM clock-gate window (3.4us) never sees an idle gap and real
   matmuls run at 2.4GHz from the start.
 - Last o-tile runs half-width matmuls and two independent all-DVE
   latency chains (per batch half), h0 postprocess overlapping h1
   matmuls.  Output DMAs ride the otherwise-idle Sync queue.
"""

import numpy as np
import ml_dtypes
from contextlib import ExitStack

import concourse.bass as bass
import concourse.mybir as mybir
import concourse.tile as tile

B, N, D, S = 64, 1024, 8, 16
NCORES = 8
BC = B // NCORES          # batches per core = 8
DS = D * S                # 128
P = 128                   # SBUF partitions
KP = 4                    # contraction pair-steps (256 input neurons each)
OT = N // P               # 8 output-neuron tiles
FH = 512                  # one fp32 PSUM bank of moving free dim
FW = 2 * FH               # full o-tile moving width (2 banks)
BD = BC * D               # 64
GRP = 4                   # o-tiles in the pair-outer leading group
B0, W1, B1 = 0, OT * D, 2 * OT * D      # smallp cols: b_dend | w_dend | b_soma
SPC = 2 * OT * D + OT                   # 136

F32 = mybir.dt.float32
BF16 = mybir.dt.bfloat16
F8 = mybir.dt.float8e4

_NC_CACHE = {}


def legalize_waits(nc, max_attached=1):
    """Split multi-semaphore waits onto preceding same-engine NOPs.

    The walrus build in this environment accepts at most one sync-wait
    command per instruction (setupSyncWait: "Too many sync wait commands"),
    but Tile attaches one wait per out-of-date engine clock.  An engine is
    in-order, so hoisting the extra waits onto NOPs immediately before the
    instruction is semantics-preserving.
    """
    nid = 0
    for f in nc.m.functions:
        for blk in f.blocks:
            new = []
            changed = False
            for inst in blk.instructions:
                si = inst.sync_info
                if si is not None and si.on_wait and len(si.on_wait) > max_attached:
                    waits = list(si.on_wait)
                    for w in waits[:-max_attached]:
                        nid += 1
                        nop = mybir.InstNoOp(name=f"WSPLIT-{nid}", ins=[], outs=[])
                        nop.engine = inst.engine
                        nop.sync_info = mybir.SyncInfo(on_wait=[w], on_update=[])
                        new.append(nop)
                    inst.sync_info = mybir.SyncInfo(
                        on_wait=waits[-max_attached:], on_update=list(si.on_update)
                    )
                    changed = True
                new.append(inst)
            if changed:
                blk.instructions = new
    return nc


def build_nc(legalize=True):
    """Build the single-core Bass program (SPMD: same program on all cores)."""
    nc = bass.Bass()
    # mt cols: (o-tile t, pair-member j, o-within-tile) so per-o-tile lhsT
    # slices and the tile-0-first DMA split are both contiguous.
    mt = nc.declare_dram_parameter("mt", [KP * P, OT * 2 * P], F8, isOutput=False)
    # xc cols: (pair-member j, (b, d, s)).
    xc = nc.declare_dram_parameter("xc", [KP * P, 2 * FW], F8, isOutput=False)
    wsyn = nc.declare_dram_parameter("wsyn", [P, OT * DS], BF16, isOutput=False)
    smallp = nc.declare_dram_parameter("smallp", [P, SPC], F32, isOutput=False)
    out = nc.declare_dram_parameter("out", [P, OT * BC], F32, isOutput=True)

    AF = mybir.ActivationFunctionType
    AX = mybir.AxisListType
    OP = mybir.AluOpType
    DR = mybir.MatmulPerfMode.DoubleRow

    with tile.TileContext(nc) as tc, ExitStack() as ctx:
        wpool = ctx.enter_context(tc.tile_pool(name="weights", bufs=1))
        xpool = ctx.enter_context(tc.tile_pool(name="xin", bufs=1))
        pspool = ctx.enter_context(tc.tile_pool(name="ps", bufs=4, space="PSUM"))
        smpool = ctx.enter_context(tc.tile_pool(name="smp", bufs=2))

        # --- PE pre-warm + ACT table preload while the first input chunk
        # is in flight.  Staggered dummies (short then long) keep the PE
        # active from ~8us until the first data lands ~12us, so the HAM
        # clock-gate lifts to 2.4GHz and never drops back. ---
        warm_sb = wpool.tile([P, FH], BF16, tag="warm", name="warm_sb")
        nc.gpsimd.memset(warm_sb[:], 0.0)
        preld = wpool.tile([P, 2], F32, tag="preld", name="preld")
        nc.scalar.activation(preld[:, 0:1], warm_sb[:, 0:1], AF.Tanh)
        nc.scalar.activation(preld[:, 1:2], warm_sb[:, 0:1], AF.Sigmoid)
        warm_ps = pspool.tile([P, FW], F32, tag="ps", name="warm_ps")
        for _ in range(8):
            nc.tensor.matmul(
                warm_ps[:, 0:P], lhsT=warm_sb[:, 0:P], rhs=warm_sb[:, 0:P],
                start=True, stop=True,
            )
        for _ in range(5):
            nc.tensor.matmul(
                warm_ps[:, 0:FH], lhsT=warm_sb[:, 0:P], rhs=warm_sb[:],
                start=True, stop=True,
            )

        # --- input DMAs: x on Sync HWDGE, mt+params on Scalar HWDGE
        # (parallel rings).  Pair 0 split so the first matmul starts after
        # ~160KB; params interleaved so they land before the first drain
        # without delaying the pair stream's critical chunks. ---
        x_tiles, mt_tiles = [], []
        for p in range(KP):
            xt = xpool.tile([P, 2 * FW], F8, tag=f"x{p}", name=f"x{p}")
            mtp = xpool.tile([P, OT * 2 * P], F8, tag=f"m{p}", name=f"m{p}")
            x_tiles.append(xt)
            mt_tiles.append(mtp)
        smallp_sb = wpool.tile([P, SPC], F32, tag="smallp", name="smallp_sb")
        wsyn_sb = wpool.tile([P, OT * DS], BF16, tag="wsyn", name="wsyn_sb")

        nc.sync.dma_start(x_tiles[0][:, 0:FW], xc[0:P, 0:FW])
        nc.sync.dma_start(x_tiles[0][:, FW:], xc[0:P, FW:])
        nc.scalar.dma_start(mt_tiles[0][:, 0:2 * P], mt[0:P, 0:2 * P])
        nc.scalar.dma_start(mt_tiles[0][:, 2 * P:], mt[0:P, 2 * P:])
        for p in range(1, KP):
            nc.sync.dma_start(x_tiles[p][:], xc[p * P:(p + 1) * P, :])
            nc.scalar.dma_start(mt_tiles[p][:], mt[p * P:(p + 1) * P, :])
            if p == 1:
                nc.scalar.dma_start(smallp_sb[:], smallp[:, :])
            elif p == 2:
                nc.sync.dma_start(wsyn_sb[:], wsyn[:, :])

        # bf16 prod for the DVE path (halves the SBUF multiply cost via the
        # 2x packed mode); separate fp32 prods for the GpSimd-multiplied
        # tiles (Q7 runs bf16 ~3x slower than fp32).
        prod_all = wpool.tile([P, OT * BC * DS], BF16, tag="prod", name="prod_all")
        prod5 = wpool.tile([P, BC * DS], F32, tag="prod5", name="prod5")
        prod6 = wpool.tile([P, BC * DS], F32, tag="prod6", name="prod6")
        dp_all = wpool.tile([P, OT * BD], F32, tag="dp", name="dp_all")
        dend_all = wpool.tile([P, OT * BD], F32, tag="dend", name="dend_all")
        soma_all = wpool.tile([P, OT * BC], F32, tag="soma", name="soma_all")
        out_sb = wpool.tile([P, OT * BC], F32, tag="out", name="out_sb")

        def mm_half(pst, t, p, h):
            nc.tensor.matmul(
                pst[:, h * FH:(h + 1) * FH],
                lhsT=mt_tiles[p][:, t * 2 * P:(t + 1) * 2 * P]
                .rearrange("p (j o) -> p j o", j=2),
                rhs=x_tiles[p][:].rearrange("p (j c) -> p j c", j=2)
                [:, :, h * FH:(h + 1) * FH],
                start=(p == 0), stop=(p == KP - 1), perf_mode=DR,
            )

        def wsyn_bc(t, b):
            return (wsyn_sb[:, t * DS:(t + 1) * DS].unsqueeze(1)
                    .broadcast_to([P, b, DS]))

        def prslice(t, b=BC, h=0):
            c0 = t * BC * DS + h * (BC // 2) * DS
            return prod_all[:, c0:c0 + b * DS]

        def drain_dve(t, pst, b=BC, h=0):
            # prod[o, b, (d,s)] = psum * w_syn (broadcast over b)
            nc.vector.tensor_mul(
                prslice(t, b, h).rearrange("p (b q) -> p b q", b=b),
                pst.rearrange("p (b q) -> p b q", b=b),
                wsyn_bc(t, b),
            )

        def act_copy(t, pst, cpy):
            # ACT copies PSUM->SBUF (it is the only PSUM-capable engine
            # besides DVE, and this frees the banks early for the trailing
            # tiles' matmuls).
            nc.scalar.activation(cpy[:], pst, AF.Copy)

        def mult_dve(t, cpy):
            # bf16 x bf16 -> bf16 tensor_tensor qualifies for the DVE 2x
            # packed mode.
            nc.vector.tensor_mul(
                prslice(t).rearrange("p (b q) -> p b q", b=BC),
                cpy[:].rearrange("p (b q) -> p b q", b=BC),
                wsyn_bc(t, BC),
            )

        def mult_gps(t, cpy, pr):
            nc.gpsimd.tensor_mul(
                pr[:].rearrange("p (b q) -> p b q", b=BC),
                cpy[:].rearrange("p (b q) -> p b q", b=BC),
                wsyn_bc(t, BC),
            )

        def sred_dve(t_lo, nt, src=None):
            if src is None:
                src = (prod_all[:, t_lo * BC * DS:(t_lo + nt) * BC * DS]
                       .rearrange("p (bd s) -> p bd s", s=S))
            nc.vector.tensor_reduce(
                dp_all[:, t_lo * BD:(t_lo + nt) * BD],
                src, axis=AX.X, op=OP.add,
            )

        def sred_gps(tag, t, pr):
            # single-tile fp32 pairwise tree on GpSimd (no free-dim
            # tensor_reduce on Q7; fp32 is its only fast dtype)
            pv = pr[:].rearrange("p (bd s) -> p bd s", s=S)
            g1 = smpool.tile([P, BD * 8], F32, tag=f"{tag}g1", name=f"{tag}g1")
            g2 = smpool.tile([P, BD * 4], F32, tag=f"{tag}g2", name=f"{tag}g2")
            g3 = smpool.tile([P, BD * 2], F32, tag=f"{tag}g3", name=f"{tag}g3")
            g1v = g1[:].rearrange("p (bd s) -> p bd s", s=8)
            g2v = g2[:].rearrange("p (bd s) -> p bd s", s=4)
            g3v = g3[:].rearrange("p (bd s) -> p bd s", s=2)
            nc.gpsimd.tensor_add(g1v, pv[:, :, 0:8], pv[:, :, 8:16])
            nc.gpsimd.tensor_add(g2v, g1v[:, :, 0:4], g1v[:, :, 4:8])
            nc.gpsimd.tensor_add(g3v, g2v[:, :, 0:2], g2v[:, :, 2:4])
            nc.gpsimd.tensor_add(
                dp_all[:, t * BD:(t + 1) * BD].unsqueeze(2),
                g3v[:, :, 0:1], g3v[:, :, 1:2])

        def bias_dve(t_lo, nt):
            dpv = (dp_all[:, t_lo * BD:(t_lo + nt) * BD]
                   .rearrange("p (t b d) -> p t b d", b=BC, d=D))
            nc.vector.tensor_add(
                dpv, dpv,
                smallp_sb[:, B0 + t_lo * D:B0 + (t_lo + nt) * D]
                .rearrange("p (t d) -> p t d", d=D).unsqueeze(2)
                .broadcast_to([P, nt, BC, D]))

        def tanh_batch(t_lo, t_hi):
            nc.scalar.activation(
                dend_all[:, t_lo * BD:t_hi * BD],
                dp_all[:, t_lo * BD:t_hi * BD], AF.Tanh)

        def soma_dve(t_lo, nt):
            # sp = dend * w_dend (broadcast over b); d-reduce; + b_soma
            # (folded so the sigmoids can batch across tiles).
            n = nt * BD
            sp = smpool.tile([P, n], F32, tag="sp", name=f"sp{t_lo}")
            nc.vector.tensor_mul(
                sp[:].rearrange("p (t b d) -> p t b d", b=BC, d=D),
                dend_all[:, t_lo * BD:(t_lo + nt) * BD]
                .rearrange("p (t b d) -> p t b d", b=BC, d=D),
                smallp_sb[:, W1 + t_lo * D:W1 + (t_lo + nt) * D]
                .rearrange("p (t d) -> p t d", d=D).unsqueeze(2)
                .broadcast_to([P, nt, BC, D]))
            ssl = slice(t_lo * BC, (t_lo + nt) * BC)
            nc.vector.tensor_reduce(
                soma_all[:, ssl],
                sp[:].rearrange("p (tb d) -> p tb d", d=D),
                axis=AX.X, op=OP.add)
            nc.vector.tensor_add(
                soma_all[:, ssl].rearrange("p (t b) -> p t b", b=BC),
                soma_all[:, ssl].rearrange("p (t b) -> p t b", b=BC),
                smallp_sb[:, B1 + t_lo:B1 + t_lo + nt].unsqueeze(2)
                .broadcast_to([P, nt, BC]))

        def sigmoid_batch(t_lo, t_hi):
            nc.scalar.activation(
                out_sb[:, t_lo * BC:t_hi * BC],
                soma_all[:, t_lo * BC:t_hi * BC], AF.Sigmoid)

        # ---- matmuls ----
        pst = {}
        for t in range(GRP):
            pst[t] = pspool.tile([P, FW], F32, tag="ps", name=f"ps{t}")
        for p in range(KP):
            for t in range(GRP):
                mm_half(pst[t], t, p, 0)
                mm_half(pst[t], t, p, 1)
        for t in range(GRP, OT):
            pst[t] = pspool.tile([P, FW], F32, tag="ps", name=f"ps{t}")
            if t < OT - 1:
                for p in range(KP):
                    mm_half(pst[t], t, p, 0)
                    mm_half(pst[t], t, p, 1)
            else:
                for h in range(2):
                    for p in range(KP):
                        mm_half(pst[t], t, p, h)

        # ---- postprocess ----
        # (Tile's per-engine scheduler reorders by readiness, so issue
        # order here only fixes relative priority.)
        cp1 = smpool.tile([P, BC * DS], BF16, tag="cpb", name="cp1", bufs=2)
        cp3 = smpool.tile([P, BC * DS], BF16, tag="cpb", name="cp3", bufs=2)
        cp5 = smpool.tile([P, BC * DS], F32, tag="cpf", name="cp5", bufs=2)
        cp6 = smpool.tile([P, BC * DS], F32, tag="cpf", name="cp6", bufs=2)
        # Leading tiles 0-3: even tiles DVE-drained; odd tiles ACT-copied
        # (bf16) then DVE-multiplied at the 2x packed rate.  Everything
        # downstream of the copies stays on DVE, so no slow-engine
        # coupling inside the lead group.
        drain_dve(0, pst[0][:])
        act_copy(1, pst[1][:], cp1)
        drain_dve(2, pst[2][:])
        act_copy(3, pst[3][:], cp3)
        mult_dve(1, cp1)
        mult_dve(3, cp3)
        sred_dve(0, 2)
        sred_dve(2, 2)
        bias_dve(0, 4)
        tanh_batch(0, 4)
        soma_dve(0, 4)
        sigmoid_batch(0, 4)
        # Trailing tile 4: ACT-copied, bf16 DVE multiply, DVE reduce.
        cp4 = smpool.tile([P, BC * DS], BF16, tag="cpb", name="cp4", bufs=2)
        act_copy(4, pst[4][:], cp4)
        mult_dve(4, cp4)
        sred_dve(4, 1)
        bias_dve(4, 1)
        tanh_batch(4, 5)
        soma_dve(4, 1)
        sigmoid_batch(4, 5)
        nc.sync.dma_start(out[:, 0:5 * BC], out_sb[:, 0:5 * BC])
        # Tiles 5-6 are ACT-copied now (frees their PSUM banks) but their
        # slow GpSimd pipelines are ISSUED AFTER tile 7's latency chains:
        # the static per-engine schedule follows program order, so nothing
        # that waits on GpSimd may precede the critical final chain.
        act_copy(5, pst[5][:], cp5)
        act_copy(6, pst[6][:], cp6)
        # Last tile: two independent all-DVE latency chains (one per
        # PSUM half = batch half).
        t = OT - 1
        HB = BC // 2          # 4 batches per half
        for h in range(2):
            drain_dve(t, pst[t][:, h * FH:(h + 1) * FH], b=HB, h=h)
            dsl = slice(t * BD + h * HB * D, t * BD + (h + 1) * HB * D)
            nc.vector.tensor_reduce(
                dp_all[:, dsl],
                prslice(t, HB, h).rearrange("p (bd s) -> p bd s", s=S),
                axis=AX.X, op=OP.add)
            nc.vector.tensor_add(
                dp_all[:, dsl].rearrange("p (b d) -> p b d", d=D),
                dp_all[:, dsl].rearrange("p (b d) -> p b d", d=D),
                smallp_sb[:, B0 + t * D:B0 + (t + 1) * D].unsqueeze(1)
                .broadcast_to([P, HB, D]))
            nc.scalar.activation(dend_all[:, dsl], dp_all[:, dsl], AF.Tanh)
            sp7 = smpool.tile([P, HB * D], F32, tag="sp7", name=f"sp7{h}")
            nc.vector.tensor_mul(
                sp7[:].rearrange("p (b d) -> p b d", d=D),
                dend_all[:, dsl].rearrange("p (b d) -> p b d", d=D),
                smallp_sb[:, W1 + t * D:W1 + (t + 1) * D].unsqueeze(1)
                .broadcast_to([P, HB, D]))
            ssl = slice(t * BC + h * HB, t * BC + (h + 1) * HB)
            nc.vector.tensor_reduce(
                soma_all[:, ssl],
                sp7[:].rearrange("p (b d) -> p b d", d=D),
                axis=AX.X, op=OP.add)
            nc.vector.tensor_add(
                soma_all[:, ssl], soma_all[:, ssl],
                smallp_sb[:, B1 + t:B1 + t + 1].broadcast_to([P, HB]))
            nc.scalar.activation(out_sb[:, ssl], soma_all[:, ssl], AF.Sigmoid)

        # Tiles 5-6 postprocess (GpSimd multiply + tree, fp32 its fast
        # dtype) — off the critical path, only their own out DMA waits.
        mult_gps(5, cp5, prod5)
        sred_gps("u", 5, prod5)
        bias_dve(5, 1)
        tanh_batch(5, 6)
        soma_dve(5, 1)
        mult_gps(6, cp6, prod6)
        sred_gps("v", 6, prod6)
        bias_dve(6, 1)
        tanh_batch(6, 7)
        soma_dve(6, 1)
        sigmoid_batch(5, 7)
        nc.sync.dma_start(out[:, 5 * BC:7 * BC], out_sb[:, 5 * BC:7 * BC])
        nc.sync.dma_start(out[:, 7 * BC:], out_sb[:, 7 * BC:])

    if legalize:
        legalize_waits(nc)
    return nc


def get_nc():
    if "nc" not in _NC_CACHE:
        _NC_CACHE["nc"] = build_nc()
    return _NC_CACHE["nc"]


def pack_static(matriz_conexao, w_syn, b_dend, w_dend, b_soma):
    """Pack the batch-independent operands (shared by all cores)."""
    # mt rows (pair p, r), cols (t, j, o): lhsT[r, j, o] = M[t*128+o, (2p+j)*128+r]
    mtT = np.ascontiguousarray(np.asarray(matriz_conexao, np.float32).T)  # [i, o]
    mt_np = (mtT.reshape(KP, 2, P, OT, P)        # [p, j, r, t, o]
             .transpose(0, 2, 3, 1, 4)           # [p, r, t, j, o]
             .reshape(KP * P, OT * 2 * P)
             .astype(ml_dtypes.float8_e4m3))
    ws = (np.asarray(w_syn, np.float32).reshape(OT, P, DS).transpose(1, 0, 2)
          .reshape(P, OT * DS).astype(ml_dtypes.bfloat16))
    bd = np.asarray(b_dend, np.float32).reshape(OT, P, D).transpose(1, 0, 2).reshape(P, OT * D)
    wd = np.asarray(w_dend, np.float32).reshape(OT, P, D).transpose(1, 0, 2).reshape(P, OT * D)
    bs = np.asarray(b_soma, np.float32).reshape(OT, P).T
    smallp_np = np.ascontiguousarray(
        np.concatenate([bd, wd, bs], axis=1).astype(np.float32))
    return mt_np, np.ascontiguousarray(ws), smallp_np


def prepare_in_maps(x, matriz_conexao, w_syn, b_dend, w_dend, b_soma):
    mt_np, ws_np, smallp_np = pack_static(matriz_conexao, w_syn, b_dend, w_dend, b_soma)
    x = np.asarray(x, np.float32)
    xq = x.astype(ml_dtypes.float8_e4m3)
    # xt[i, b, (d,s)] then per core rows (pair p, r), cols (j, b, d, s)
    xt = np.ascontiguousarray(xq.transpose(1, 0, 2, 3).reshape(N, B, DS))
    in_maps = []
    for c in range(NCORES):
        xcor = xt[:, c * BC:(c + 1) * BC, :]          # [N, 8, 128]
        xc_np = np.ascontiguousarray(
            xcor.reshape(KP, 2, P, BC * DS)            # [p, j, r, c]
            .transpose(0, 2, 1, 3)                     # [p, r, j, c]
            .reshape(KP * P, 2 * FW))
        in_maps.append({"mt": mt_np, "xc": xc_np,
                        "wsyn": ws_np, "smallp": smallp_np})
    return in_maps


def assemble_output(results):
    outs = []
    for c in range(NCORES):
        oc = np.asarray(results[c]["out"])          # [P, (t, b)]
        outs.append(oc.reshape(P, OT, BC).transpose(2, 1, 0).reshape(BC, N))
    return np.ascontiguousarray(np.concatenate(outs, axis=0).astype(np.float32))


def kernel(x, matriz_conexao, w_syn, b_dend, w_dend, b_soma):
    from concourse.bass_utils import run_bass_kernel_spmd
    in_maps = prepare_in_maps(x, matriz_conexao, w_syn, b_dend, w_dend, b_soma)
    nc = get_nc()
    res = run_bass_kernel_spmd(nc, in_maps, list(range(NCORES)))
    return assemble_output(res.results)
